# revision 51
# baseline (speedup 1.0000x reference)
"""Multi-head attention (RoPE, causal) Trainium2 Bass kernel, 8-core SPMD.

Sharding: core c = (batch b = c // 4, head-group g = c % 4); each core computes
4 of the 16 heads for one batch, including its slice of the Q/K/V projections
and a partial output projection.  The host sums the 4 partial outputs per
batch (tensor-parallel unshard).

Device layout notes:
  - x is pre-transposed on host to xT [D, S] so projection matmuls contract
    over D on partitions.
  - Wq/Wk rows are host-permuted so the projection PSUM M-tiles are directly
    the RoPE operand layouts: tile0 = even ("x1") dims of all 4 heads
    stacked [h0(32) h1 h2 h3], tile1 = odd ("x2") dims.  RoPE is then pure
    lane-aligned elementwise DVE work reading PSUM directly, producing
    [FH;SH] stacks that a PE pair-shuffle converts to per-head K=64 layouts.
  - scores are computed transposed, scoresT[sk, sq], one PSUM bank per head
    (row strips 32h -> concurrent matmuls), softmax runs without max
    subtraction as exp(s/8 - 8) (exact: constant shift), the denominator is a
    col-packed M=1 ones-matmul per head, and AV needs no transposes:
    out_hT[dh, sq] = v_h[sk, dh].T @ expT[sk, sq].
  - normalization: the denominator row is copied to SBUF (custom-DVE ops
    cannot read PSUM on hardware), reciprocal'd, gpsimd-broadcast to 64
    partitions, and one DVE mul (PSUM x SBUF) writes the bf16 attT tile.
  - causal structure is exploited generally: the host classifies every
    (sq-block 512 x sk-tile 128) mask block as skip / full / pattern and the
    kernel only emits work for non-skip blocks, narrowing columns to the
    non-masked range.  Patterns (0/1) multiply the exp tile - exact.
  - startup: junk N=64 matmuls warm the PE clock (HAM) while weights stream;
    DRAM params are stored partition-major (4KB DMA descriptors); x blocks
    load as two half-tiles so consumers start on the first half.
  - scheduling: proj(B+1) pieces interleave into att(B) as paced fillers,
    2 held back per pass end to cover the softmax-normalize latency; Wo(0)
    runs inside att(2), Wo(1)+Wo(2) inside att(3) where exp gives slack.
  - y is written bf16 per wo-pair (16 small DMAs); host accumulates in fp32.
"""

import sys

for _p in ("/opt/trn_rl_repo", "/root/.axon_site"):
    if _p not in sys.path:
        sys.path.insert(0, _p)

import numpy as np
import ml_dtypes

import concourse.bacc as bacc
import concourse.mybir as mybir
import concourse.tile as tile
from concourse.bass_utils import run_bass_kernel_spmd

BF16 = mybir.dt.bfloat16
F16 = mybir.dt.float16
F32 = mybir.dt.float32
NP_BF16 = ml_dtypes.bfloat16

# Problem constants (hardcoded per contract)
B, S, D = 2, 2048, 1024
H, DH = 16, 64
ROPE_BASE = 10000.0
NCORES = 8
GROUPS = 4            # head-groups per batch
HPC = H // GROUPS     # 4 heads per core
DC = HPC * DH         # 256 head dims per core
SB = 512              # sq block
NSB = S // SB         # 4 sq blocks
SK = 128              # sk tile
NSK = S // SK         # 16 sk tiles
DT = D // 128         # 8 di tiles
SCALE = 1.0 / np.sqrt(DH)
EXP_SHIFT = -4.0          # keeps denominators well-scaled
N_WARM = 110          # junk matmuls bridge the PE-warm gap until weights land


# ---------------------------------------------------------------- host prep

def _rope_tables():
    """CE/SE/SO/CO [32, S] per reference's interleaved-rope formula,
    tiled x4 on partitions -> [4, 128, S] float32."""
    inv_freq = 1.0 / (ROPE_BASE ** (np.arange(0, DH, 2, dtype=np.float64) / DH))
    t = np.arange(S, dtype=np.float64)
    freqs = np.outer(t, inv_freq)                    # [S, 32]
    emb = np.concatenate([freqs, freqs], axis=-1)    # [S, 64]
    m = np.arange(32)
    ce = np.cos(emb[:, 2 * m]).T                     # [32, S]
    se = np.sin(emb[:, 2 * m]).T
    so = np.sin(emb[:, 2 * m + 1]).T
    co = np.cos(emb[:, 2 * m + 1]).T
    # packed for fused rope: T1=[CE;SO], T2=[-SE;CO] so FH/SH are one add
    out = np.stack([ce, so, -se, co]).astype(np.float32)   # [4, 32, S]
    return np.tile(out, (1, 4, 1))                   # [4, 128, S]


def _plan_mask(mask):
    """Classify each (sq-block, sk-tile) mask block.

    Returns (units, patterns): units[Bb] = list of (k, c0, c1, pidx|None);
    patterns = [128, 512]-padded 0/1 bf16 tiles (transposed blocks).
    Skip blocks are omitted.  Columns < c0 of a kept block are all-masked,
    columns >= c1 are all-allowed, and [c0, c1) multiplies pattern pidx."""
    units = []
    pat_idx = {}
    pats = []
    for Bb in range(NSB):
        row = []
        for k in range(NSK):
            bt = mask[SB * Bb:SB * (Bb + 1), SK * k:SK * (k + 1)].T  # [128sk, 512sq]
            bt = (bt != 0)
            any_col = bt.any(axis=0)
            if not any_col.any():
                continue
            all_col = bt.all(axis=0)
            c0 = int(np.argmax(any_col))
            not_all = np.nonzero(~all_col)[0]
            c1 = int(not_all.max()) + 1 if len(not_all) else 0
            c1 = max(c1, c0)
            pidx = None
            if c1 > c0:
                key = (c0, c1, bt[:, c0:c1].tobytes())
                if key not in pat_idx:
                    pat_idx[key] = len(pats)
                    p = np.zeros((128, 512), dtype=NP_BF16)
                    p[:, c0:c1] = bt[:, c0:c1].astype(NP_BF16)
                    pats.append(p)
                pidx = pat_idx[key]
            row.append((k, c0, c1, pidx))
        units.append(row)
    if not pats:
        pats.append(np.zeros((128, 512), dtype=NP_BF16))
    return units, np.stack(pats)


def _prep_core_inputs(x, Wq, Wk, Wv, Wo, tables, patterns, core):
    b, g = core // GROUPS, core % GROUPS
    heads = [GROUPS * g + j for j in range(HPC)]

    def pmajor(a, rows):                 # [D, cols] -> [128, D//128, cols]
        return np.ascontiguousarray(
            a.reshape(rows, 128, a.shape[-1]).transpose(1, 0, 2))

    xT = pmajor(x[b].T.astype(NP_BF16), DT)

    x1_rows = [64 * h + 2 * m for h in heads for m in range(32)]
    x2_rows = [64 * h + 2 * m + 1 for h in heads for m in range(32)]
    wq = pmajor(Wq[x1_rows + x2_rows].T.astype(NP_BF16), DT)
    wk = pmajor(Wk[x1_rows + x2_rows].T.astype(NP_BF16), DT)

    v_rows = [64 * h + d for h in heads for d in range(DH)]
    wv = pmajor(Wv[v_rows].T.astype(NP_BF16), DT)

    # att_outT partition tiles hold local heads [0,2] and [1,3]
    wo_cols = [64 * heads[j] + d for j in (0, 2, 1, 3) for d in range(DH)]
    wo = pmajor(Wo[:, wo_cols].T.astype(NP_BF16), 2)

    # PE shuffle selectors: q2/k2 pair layouts from FH/SH-stacked rope output
    shuf = np.zeros((4, 128, 128), dtype=NP_BF16)
    for p_ in range(2):
        for k_ in range(32):
            base = 64 * p_
            shuf[2 * p_, base + k_, k_] = 1          # fh head 2p -> rows 0-31
            shuf[2 * p_, base + 32 + k_, 64 + k_] = 1  # fh head 2p+1 -> 64-95
            shuf[2 * p_ + 1, base + k_, 32 + k_] = 1   # sh head 2p -> 32-63
            shuf[2 * p_ + 1, base + 32 + k_, 96 + k_] = 1  # sh head 2p+1 -> 96-127
    return {
        "xT": xT, "wq": wq, "wk": wk, "wv": wv, "wo": wo,
        "rope": np.ascontiguousarray(
            tables.astype(NP_BF16).transpose(1, 0, 2)),
        "pats": np.ascontiguousarray(patterns.transpose(1, 0, 2)),
        "shuf": np.ascontiguousarray(shuf.transpose(1, 0, 2)),
    }


# ---------------------------------------------------------------- program

_CACHE = {}


def _build(units, npat):
    nc = bacc.Bacc(None)
    xT_d = nc.declare_dram_parameter("xT", [128, DT, S], BF16, isOutput=False)
    wq_d = nc.declare_dram_parameter("wq", [128, DT, DC], BF16, isOutput=False)
    wk_d = nc.declare_dram_parameter("wk", [128, DT, DC], BF16, isOutput=False)
    wv_d = nc.declare_dram_parameter("wv", [128, DT, DC], BF16, isOutput=False)
    wo_d = nc.declare_dram_parameter("wo", [128, 2, D], BF16, isOutput=False)
    rope_d = nc.declare_dram_parameter("rope", [128, 4, S], BF16, isOutput=False)
    pats_d = nc.declare_dram_parameter("pats", [128, npat, 512], BF16, isOutput=False)
    shuf_d = nc.declare_dram_parameter("shuf", [128, 4, 128], BF16, isOutput=False)
    y_d = nc.declare_dram_parameter("y", [128, DT, S], BF16, isOutput=True)

    with tile.TileContext(nc) as tc:
        _emit(nc, tc, xT_d, wq_d, wk_d, wv_d, wo_d, rope_d, pats_d, shuf_d,
              y_d, units, npat)
    nc.compile()
    return nc


def _emit(nc, tc, xT_d, wq_d, wk_d, wv_d, wo_d, rope_d, pats_d, shuf_d,
          y_d, units, npat):
    from contextlib import ExitStack
    ctx = ExitStack()
    with ctx:
        const = ctx.enter_context(tc.tile_pool(name="const", bufs=1))
        persist = ctx.enter_context(tc.tile_pool(name="persist", bufs=1))
        work = ctx.enter_context(tc.tile_pool(name="work", bufs=4))
        xp = ctx.enter_context(tc.tile_pool(name="xp", bufs=4))
        normc = ctx.enter_context(tc.tile_pool(name="normc", bufs=2))
        expp = ctx.enter_context(tc.tile_pool(name="expp", bufs=8))
        yp = ctx.enter_context(tc.tile_pool(name="yp", bufs=4))
        # PSUM: pair tiles [128,2,SB] x3 (scores + proj filler) + 2 AV banks
        psA = ctx.enter_context(tc.tile_pool(name="psA", bufs=3, space="PSUM"))
        psAV = ctx.enter_context(tc.tile_pool(name="psAV", bufs=2, space="PSUM"))

        # ---- DMA schedule: wq+x on sync (critical path), early tables on
        # gpsimd, late bulk on the scalar queue so nothing queues behind wq.
        wq_s = persist.tile([128, DT, DC], BF16, tag="wq")
        nc.sync.dma_start(wq_s[:], wq_d[:])
        wk_s = persist.tile([128, DT, DC], BF16, tag="wk")
        nc.gpsimd.dma_start(wk_s[:], wk_d[:])
        hT = DT // 2
        x0a = xp.tile([128, hT, SB], BF16, tag="xT", name="xTa0")
        nc.sync.dma_start(x0a[:], xT_d[:, :hT, 0:SB])
        x0b = xp.tile([128, hT, SB], BF16, tag="xT", name="xTb0")
        nc.gpsimd.dma_start(x0b[:], xT_d[:, hT:, 0:SB])
        ropes = const.tile([128, 4, S], BF16, tag="rope")
        nc.gpsimd.dma_start(ropes[:, :, 0:SB], rope_d[:, :, 0:SB])
        shuf_s = const.tile([128, 4, 128], BF16, tag="shuf")
        nc.gpsimd.dma_start(shuf_s[:], shuf_d[:])
        wv_s = persist.tile([128, DT, DC], BF16, tag="wv")
        nc.gpsimd.dma_start(wv_s[:], wv_d[:])
        # bulk tables are DMA'd later on the sync queue (after x1) so the
        # startup-critical wq/x/wk transfers aren't sharing SDMA bandwidth
        pats = const.tile([128, npat, 512], BF16, tag="pats")
        wo_s = persist.tile([128, 2, D], BF16, tag="wo")

        def load_bulk():
            nc.sync.dma_start(pats[:], pats_d[:])
            for cix in range(1, 4):
                nc.sync.dma_start(ropes[:, :, SB * cix:SB * (cix + 1)],
                                  rope_d[:, :, SB * cix:SB * (cix + 1)])
            nc.sync.dma_start(wo_s[:], wo_d[:])

        # ---- constants
        bias8 = const.tile([128, 1], F32, tag="bias8")
        nc.vector.memset(bias8[:], EXP_SHIFT)
        warm = const.tile([128, 64], BF16, tag="warm")
        nc.vector.memset(warm[:], 0.0)

        # ---- junk matmuls: warm the PE clock (HAM) while weights stream
        ps_warm = psA.tile([128, 2, SB], F32, tag="ps", name="ps_warm")
        for _ in range(N_WARM):
            nc.tensor.matmul(ps_warm[0:1, 0, 0:64], warm[:, 0:1], warm[:, :],
                             start=True, stop=True)

        # ---- persistent per-B activations
        vSB, attT, q2, k2 = {}, {}, {}, {}
        for Bb in range(NSB):
            vSB[Bb] = persist.tile([128, 4, HPC, 65], BF16, tag=f"v{Bb}",
                                   name=f"v{Bb}")  # 4 sk tiles, per-head [v|1]
            attT[Bb] = persist.tile([128, 2, SB], BF16, tag=f"att{Bb}",
                                    name=f"att{Bb}")
            q2[Bb] = persist.tile([128, 2, SB], BF16, tag=f"q2{Bb}", name=f"q2{Bb}")
            k2[Bb] = persist.tile([128, 2, SB], BF16, tag=f"k2{Bb}", name=f"k2{Bb}")

        def kv_tiles(k):          # global sk tile -> (block idx, col offset)
            return k // 4, (k % 4) * SK

        def load_x(Bb):
            s0 = SB * Bb
            xa = xp.tile([128, hT, SB], BF16, tag="xT", name=f"xTa{Bb}")
            nc.sync.dma_start(xa[:], xT_d[:, :hT, s0:s0 + SB])
            xb = xp.tile([128, hT, SB], BF16, tag="xT", name=f"xTb{Bb}")
            nc.sync.dma_start(xb[:], xT_d[:, hT:, s0:s0 + SB])
            return xa, xb

        def make_proj(Bb, xT, startup=False):
            """proj piece closures for block Bb: (main pieces, v pieces)."""
            s0 = SB * Bb
            ps_qk = {}

            def xslice(dt_i, cols=slice(None)):
                return xT[dt_i // hT][:, dt_i % hT, cols]

            def qk_chain(w_s, nm, mt, lo, hi):
                def go():
                    if nm not in ps_qk:
                        ps_qk[nm] = psA.tile([128, 2, SB], F32, tag="ps",
                                             name=f"{nm}ps{Bb}")
                    ps = ps_qk[nm]
                    for dt_i in range(lo, hi):
                        nc.tensor.matmul(
                            ps[:, mt, :], w_s[:, dt_i, 128 * mt:128 * (mt + 1)],
                            xslice(dt_i),
                            start=(dt_i == 0), stop=(dt_i == DT - 1))
                return go

            rope_out = {}

            def rope_piece(nm):
                def go():
                    ps = ps_qk[nm]
                    ta = work.tile([128, 2, SB], BF16, tag="ropea",
                                   name=f"{nm}ta{Bb}")
                    tb = work.tile([128, 2, SB], BF16, tag="ropeb",
                                   name=f"{nm}tb{Bb}")
                    fs = work.tile([128, 2, SB], BF16, tag="ropef",
                                   name=f"{nm}fs{Bb}")
                    nc.vector.tensor_mul(
                        ta[:], ps[:, 0:1, :].to_broadcast([128, 2, SB]),
                        ropes[:, 0:2, s0:s0 + SB])
                    nc.vector.tensor_mul(
                        tb[:], ps[:, 1:2, :].to_broadcast([128, 2, SB]),
                        ropes[:, 2:4, s0:s0 + SB])
                    nc.vector.tensor_add(fs[:], ta[:], tb[:])
                    rope_out[nm] = fs
                return go

            def shuf_piece(nm):
                def go():
                    fs = rope_out[nm]
                    # pair shuffle on PE -> [fh;sh]-per-head K=64 layout
                    t2 = q2[Bb] if nm == "q" else k2[Bb]
                    ps2 = psA.tile([128, 2, SB], F32, tag="ps", name=f"{nm}s2{Bb}")
                    for p_ in range(2):
                        nc.tensor.matmul(ps2[:, p_, :], shuf_s[:, 2 * p_, :],
                                         fs[:, 0, :], start=True, stop=False)
                        nc.tensor.matmul(ps2[:, p_, :], shuf_s[:, 2 * p_ + 1, :],
                                         fs[:, 1, :], start=False, stop=True)
                    nc.vector.tensor_copy(t2[:], ps2[:])
                return go

            def v_piece(pp):
                def go():
                    # halves on separate PSUM banks (8*64 fp32 = one bank)
                    ps = psA.tile([128, 2, 8, 64], F32, tag="ps",
                                  name=f"vps{Bb}_{pp}")
                    for half in range(2):
                        ck = 2 * pp + half
                        for dt_i in range(DT):
                            nc.tensor.matmul(
                                ps[:, half, 0:HPC, :],
                                xslice(dt_i, slice(128 * ck, 128 * (ck + 1))),
                                wv_s[:, dt_i, :],
                                start=(dt_i == 0), stop=(dt_i == DT - 1))
                    for half in range(2):
                        ck = 2 * pp + half
                        nc.vector.tensor_copy(vSB[Bb][:, ck, :, 0:64],
                                              ps[:, half, 0:HPC, :])
                    if pp == 1:
                        nc.vector.memset(vSB[Bb][:, :, :, 64], 1.0)
                return go

            if startup:
                main = [qk_chain(wq_s, "q", 0, 0, hT), qk_chain(wq_s, "q", 1, 0, hT),
                        qk_chain(wq_s, "q", 0, hT, DT), qk_chain(wq_s, "q", 1, hT, DT),
                        qk_chain(wk_s, "k", 0, 0, DT), qk_chain(wk_s, "k", 1, 0, DT),
                        rope_piece("q"), v_piece(0), shuf_piece("q"),
                        rope_piece("k"), v_piece(1), shuf_piece("k")]
                return main, []
            main = [qk_chain(wq_s, "q", 0, 0, DT), qk_chain(wq_s, "q", 1, 0, DT),
                    rope_piece("q"), shuf_piece("q"),
                    qk_chain(wk_s, "k", 0, 0, DT), qk_chain(wk_s, "k", 1, 0, DT),
                    rope_piece("k"), shuf_piece("k")]
            vs = [v_piece(0), v_piece(1)]
            return main, vs

        def wo_pieces(Bb, on_act=False):
            s0 = SB * Bb

            def pair(pp):
                def go():
                    ps = psA.tile([128, 2, SB], F32, tag="ps",
                                  name=f"yps{Bb}_{pp}")
                    for half in range(2):
                        t = 2 * pp + half
                        for c in range(2):
                            nc.tensor.matmul(
                                ps[:, half, :],
                                wo_s[:, c, 128 * t:128 * (t + 1)],
                                attT[Bb][:, c, :],
                                start=(c == 0), stop=(c == 1))
                    yt = yp.tile([128, 2, SB], BF16, tag="yt",
                                 name=f"yt{Bb}_{pp}")
                    if on_act and pp % 2 == 0:   # split tail casts ACT/DVE
                        nc.scalar.copy(yt[:], ps[:])
                    else:
                        nc.vector.tensor_copy(yt[:], ps[:])
                    nc.sync.dma_start(y_d[:, 2 * pp:2 * pp + 2, s0:s0 + SB],
                                      yt[:])
                return go
            return [pair(pp) for pp in range(DT // 2)]

        def att_block(Bb, front=(), filler=()):
            front = list(front)
            filler = list(filler)
            row = units[Bb]
            if not row:
                nc.vector.memset(attT[Bb][:], 0.0)
                for f in front + filler:
                    f()
                return
            # front pieces run one-per-slot; the rest pace over the slots,
            # holding back 2 pieces per pass end to cover the normalize
            # latency (keeps the PE fed and HAM warm at block boundaries)
            slots = 2 * len(row)
            state = {"ends": 2}
            n_paced = max(0, len(filler) - 2 * state["ends"])
            nf = (max(1, (slots - len(front)) // n_paced) if n_paced else 0)
            slot = 0

            def pop_filler():
                if front:
                    front.pop(0)()
                elif (filler and len(filler) > 2 * state["ends"]
                      and nf and slot % nf == nf - 1):
                    filler.pop(0)()

            for p in range(2):                     # head-pair pass
                av = [psAV.tile([128, SB], F32, tag="av", name=f"av{Bb}_{p}_{h}")
                      for h in range(2)]

                def emit_scores(ui):
                    k, c0, c1, pidx = row[ui]
                    kb, ko = kv_tiles(k)
                    sc = psA.tile([128, 2, SB], F32, tag="ps",
                                  name=f"sc{Bb}_{p}_{ui}")
                    for half in range(2):
                        nc.tensor.matmul(
                            sc[:, half, c0:],
                            k2[kb][64 * half:64 * (half + 1), p, ko:ko + SK],
                            q2[Bb][64 * half:64 * (half + 1), p, c0:],
                            start=True, stop=True,
                            tile_position=(64 * half, 0))
                    ex = expp.tile([128, 2, SB], BF16, tag="exp",
                                   name=f"ex{Bb}_{p}_{ui}")
                    if ui == 0 and c0 > 0:
                        nc.vector.memset(ex[:], 0.0)
                    nc.scalar.activation(
                        ex[:, :, c0:], sc[:, :, c0:],
                        mybir.ActivationFunctionType.Exp,
                        bias=bias8[:], scale=SCALE)
                    if pidx is not None:
                        w = c1 - c0
                        nc.vector.tensor_mul(
                            ex[:, :, c0:c1], ex[:, :, c0:c1],
                            pats[:, pidx:pidx + 1, c0:c1].to_broadcast([128, 2, w]))
                    return ex

                def emit_av(ui, ex):
                    k, c0, c1, pidx = row[ui]
                    kb, ko = kv_tiles(k)
                    first, last = ui == 0, ui == len(row) - 1
                    w0 = 0 if first else c0        # accum write start col
                    for half in range(2):
                        j = 2 * p + half
                        nc.tensor.matmul(
                            av[half][0:65, w0:],
                            vSB[kb][:, ko // SK, j, :],
                            ex[:, half, w0:],
                            start=first, stop=last)

                prev = None
                for ui in range(len(row)):
                    ex = emit_scores(ui)
                    if prev is not None:
                        emit_av(prev[0], prev[1])
                    pop_filler()
                    slot += 1
                    prev = (ui, ex)
                emit_av(prev[0], prev[1])

                # normalize: denominator row -> SBUF (custom-DVE recip can't
                # read PSUM), gpsimd broadcast, one mul -> bf16 attT
                rcs = []
                for half in range(2):
                    den = normc.tile([1, SB], F32, tag="den",
                                     name=f"den{Bb}_{p}_{half}")
                    nc.vector.tensor_copy(den[:], av[half][64:65, :])
                    rc = normc.tile([1, SB], F32, tag="recip",
                                    name=f"rc{Bb}_{p}_{half}")
                    nc.vector.reciprocal_approx_fast(out=rc[:], in_=den[:])
                    rcs.append(rc)
                state["ends"] -= 1
                for _ in range(2):                 # cover the recip latency
                    if front:
                        front.pop(0)()
                    elif filler:
                        filler.pop(0)()
                rcbs = []
                for half in range(2):
                    rcb = normc.tile([64, SB], F32, tag="rcb",
                                     name=f"rcb{Bb}_{p}_{half}")
                    nc.gpsimd.partition_broadcast(rcb[:], rcs[half][0:1, :])
                    rcbs.append(rcb[:])
                for half in range(2):
                    j = 2 * p + half               # head j -> ptile j%2, rows 64*(j//2)
                    bank, rhalf = j % 2, j // 2
                    nc.vector.tensor_mul(
                        attT[Bb][64 * rhalf:64 * (rhalf + 1), bank, :],
                        av[half][0:64, :], rcbs[half])
            for f in front + filler:
                f()

        # ---- schedule: proj(B+1) and Wo(B-1) interleave into att(B)
        xT0 = (x0a, x0b)
        main0, _ = make_proj(0, xT0, startup=True)
        for f in main0:
            f()
        xT1 = load_x(1)
        load_bulk()
        main1, vs1 = make_proj(1, xT1)
        att_block(0, filler=main1 + vs1)
        xT2 = load_x(2)
        main2, vs2 = make_proj(2, xT2)
        att_block(1, filler=main2 + vs2)
        xT3 = load_x(3)
        main3, vs3 = make_proj(3, xT3)
        att_block(2, filler=main3 + wo_pieces(0))
        att_block(3, front=vs3, filler=wo_pieces(1) + wo_pieces(2))
        for f in wo_pieces(3, on_act=True):
            f()


# ---------------------------------------------------------------- entry

def _get_program(mask):
    key = mask.tobytes()
    if key not in _CACHE:
        units, patterns = _plan_mask(np.asarray(mask))
        nc = _build(units, patterns.shape[0])
        _CACHE[key] = (nc, units, patterns)
    return _CACHE[key]


def kernel(x, Wq, Wk, Wv, Wo, attn_mask, _trace=False):
    x = np.asarray(x, dtype=np.float32)
    Wq, Wk, Wv, Wo = (np.asarray(w, dtype=np.float32) for w in (Wq, Wk, Wv, Wo))
    attn_mask = np.asarray(attn_mask)

    nc, units, patterns = _get_program(attn_mask)
    tables = _rope_tables()
    in_maps = [_prep_core_inputs(x, Wq, Wk, Wv, Wo, tables, patterns, c)
               for c in range(NCORES)]
    res = run_bass_kernel_spmd(nc, in_maps, core_ids=list(range(NCORES)),
                               trace=_trace)

    out = np.zeros((B, S, D), dtype=np.float32)
    for c in range(NCORES):
        yc = np.asarray(res.results[c]["y"], dtype=np.float32)
        yT = yc.reshape(128, DT, S).transpose(1, 0, 2).reshape(D, S)
        out[c // GROUPS] += yT.T
    if _trace:
        return out, res
    return out
